# revision 1
# baseline (speedup 1.0000x reference)
"""Lookahead depthwise convolution on 8 Trainium2 NeuronCores.

out[t, b, f] = sum_{c=0..K-1} x[t+c, b, f] * weight[f, c], zero-padded at the
right edge. x: (2048, 32, 1280) fp32, weight: (1280, 81) fp32.

Strategy: shard the (fully independent) feature dim across 8 cores, 160
features each. Per feature the time conv is a banded Toeplitz matmul: with
128-wide time tiles, out_i = A_f @ x_i + B_f @ x_{i+1} where
  A_f[p, m] = w[f, p - m]        (0 <= p - m < K)
  B_f[p, m] = w[f, p + 128 - m]  (0 <= p + 128 - m < K)
Both 128x128 stationary matrices per feature are precomputed on the host in
fp16 and stay resident in SBUF. fp16 x fp16 products are exact in the fp32
PSUM accumulator, so the only error is fp16 input rounding (~1e-3 rel).
"""

import numpy as np

import concourse.bass as bass
import concourse.bacc as bacc
import concourse.mybir as mybir
from concourse import tile
from concourse.bass_utils import run_bass_kernel_spmd

S, B, F, K = 2048, 32, 1280, 81
N_CORES = 8
FC = F // N_CORES          # features per core (160)
TB = S // 128              # time blocks (16)
FPB = 16                   # features evicted per PSUM bank (16 * 32 = 512)
BAND_FREE = FC * 2 * 128   # free-dim elems of the resident band tile (40960)

_compiled = None


def _build_program():
    nc = bacc.Bacc("TRN2", target_bir_lowering=False, debug=False)
    f32, f16 = mybir.dt.float32, mybir.dt.float16

    x_in = nc.declare_dram_parameter("x", [S, B, FC], f32, isOutput=False)
    bands_in = nc.declare_dram_parameter("bands", [128, BAND_FREE], f16,
                                         isOutput=False)
    out_ext = nc.declare_dram_parameter("out", [S, B, FC], f32, isOutput=True)

    x_flat = x_in.rearrange("s b f -> s (b f)")
    out_flat = out_ext.rearrange("s b f -> s (b f)")
    BF = B * FC  # 5120

    with tile.TileContext(nc) as tc:
        with (
            tc.tile_pool(name="bands", bufs=1) as bpool,
            tc.tile_pool(name="x32", bufs=2) as x32pool,
            tc.tile_pool(name="x16", bufs=4) as x16pool,
            tc.tile_pool(name="stage", bufs=2) as spool,
            tc.tile_pool(name="psum", bufs=6, space="PSUM") as ppool,
        ):
            band_tile = bpool.tile([128, BAND_FREE], f16)
            nc.gpsimd.dma_start(out=band_tile[:], in_=bands_in[:])

            def load_block(i):
                x32 = x32pool.tile([128, BF], f32)
                nc.gpsimd.dma_start(
                    out=x32[:], in_=x_flat[i * 128:(i + 1) * 128, :])
                x16 = x16pool.tile([128, BF], f16)
                nc.scalar.copy(out=x16[:], in_=x32[:])
                return x16

            x16_cur = load_block(0)
            for i in range(TB):
                x16_nxt = load_block(i + 1) if i + 1 < TB else None
                stage = spool.tile([128, BF], f32)
                cur_v = x16_cur.rearrange("t (b f) -> t b f", f=FC)
                nxt_v = (x16_nxt.rearrange("t (b f) -> t b f", f=FC)
                         if x16_nxt is not None else None)
                for g in range(FC // FPB):          # 10 psum-bank groups
                    psum = ppool.tile([128, FPB * B], f32)
                    for j in range(FPB):
                        f = g * FPB + j
                        lA = band_tile[:, f * 256:f * 256 + 128]
                        nc.tensor.matmul(
                            out=psum[:, j * B:(j + 1) * B],
                            lhsT=lA, rhs=cur_v[:, :, f],
                            start=True, stop=(nxt_v is None))
                        if nxt_v is not None:
                            lB = band_tile[:, f * 256 + 128:f * 256 + 256]
                            nc.tensor.matmul(
                                out=psum[:, j * B:(j + 1) * B],
                                lhsT=lB, rhs=nxt_v[:, :, f],
                                start=False, stop=True)
                    # psum free layout is (j, b); stage wants (b, f)
                    psum_v = psum.rearrange("t (j b) -> t b j", j=FPB)
                    stage_v = stage.rearrange("t (b f) -> t b f", f=FC)
                    nc.vector.tensor_copy(
                        out=stage_v[:, :, g * FPB:(g + 1) * FPB],
                        in_=psum_v)
                nc.gpsimd.dma_start(
                    out=out_flat[i * 128:(i + 1) * 128, :], in_=stage[:])
                x16_cur = x16_nxt
    nc.finalize()
    return nc


def _build_bands(weight):
    p = np.arange(128)[:, None]
    m = np.arange(128)[None, :]
    dA = p - m
    dB = p + 128 - m
    mA = (dA >= 0) & (dA < K)
    mB = (dB >= 0) & (dB < K)
    iA = np.clip(dA, 0, K - 1)
    iB = np.clip(dB, 0, K - 1)
    w16 = weight.astype(np.float16).astype(np.float32)
    A = w16[:, iA] * mA          # [F, 128p, 128m]
    Bm = w16[:, iB] * mB
    bands = np.empty((128, F, 2, 128), np.float16)
    bands[:, :, 0, :] = A.transpose(1, 0, 2)
    bands[:, :, 1, :] = Bm.transpose(1, 0, 2)
    return bands


def kernel(x, weight):
    global _compiled
    x = np.asarray(x, dtype=np.float32)
    weight = np.asarray(weight, dtype=np.float32)
    if _compiled is None:
        _compiled = _build_program()
    nc = _compiled
    bands = _build_bands(weight)
    in_maps = []
    for c in range(N_CORES):
        fl = slice(c * FC, (c + 1) * FC)
        in_maps.append({
            "x": np.ascontiguousarray(x[:, :, fl]),
            "bands": np.ascontiguousarray(
                bands[:, fl, :, :]).reshape(128, BAND_FREE),
        })
    res = run_bass_kernel_spmd(nc, in_maps, list(range(N_CORES)))
    outs = [np.asarray(res.results[c]["out"]) for c in range(N_CORES)]
    return np.concatenate(outs, axis=2).astype(np.float32)



# revision 3
# speedup vs baseline: 2.0243x; 2.0243x over previous
"""Lookahead depthwise convolution on 8 Trainium2 NeuronCores.

out[t, b, f] = sum_{c=0..80} x[t+c, b, f] * weight[f, c], zero-padded at the
right edge. x: (2048, 32, 1280) fp32, weight: (1280, 81) fp32.

Feature-sharded across 8 cores (160 features each; the conv is depthwise so
features are fully independent). Per feature the time conv is a banded
Toeplitz matmul: with 128-wide time tiles,
  out_i = A_f^T x_i + B_f^T x_{i+1}
  A_f[m, p] = w[f, m - p]        (0 <= m - p <= 80)   [128 x 128]
  B_f[m, p] = w[f, m + 128 - p]  (0 <= m+128-p <= 80) [80 x 128, rows m<80]

Key layout trick: the host transposes x to feature-major panels
x_panel[f, m, i*32 + b] = x[i*128 + m, b, f] so each feature's ENTIRE time
series is one contiguous [128, 512] fp16 tile. One weight load then streams
512 columns (all 16 time blocks x 32 batch) instead of 32 — the stationary
band is loaded once per feature instead of once per (feature, time block),
keeping the PE streaming-bound instead of LDWEIGHTS-bound. The B band
accumulates the next-block contribution into the same PSUM bank at a
32-column offset; its contraction is trimmed to the 80 nonzero rows.

All device I/O is fp16 (inputs rounded on host, outputs upcast on host);
fp32 PSUM accumulation keeps rel err ~5e-4, well under the 2e-2 gate. The
kernel is HBM-read-stream bound: 21 MB x + 8.3 MB bands per core in, 21 MB
out per core out, with reads and writes on independent paths.

Features are processed in groups of 16 so every x/out DMA is a 1-2 MB
contiguous transfer (>90% DMA efficiency); x-in and out DMAs are split in
halves so the first matmul starts after half a group and the last group's
out-DMA overlaps its evictions. PSUM->SBUF eviction alternates between the
Vector and Scalar engines so neither becomes the bottleneck; in-DMAs ride
the SP HWDGE ring while out-DMAs ride the Activation ring.
"""

import numpy as np

import concourse.bass as bass
import concourse.bacc as bacc
import concourse.mybir as mybir
from concourse import tile
from concourse.bass_utils import run_bass_kernel_spmd

S, B, F, K = 2048, 32, 1280, 81
N_CORES = 8
FC = F // N_CORES      # features per core (160)
NB = S // 128          # time blocks (16)
G = 16                 # features per DMA group
NG = FC // G           # groups per core (10)
XW = G * NB * B        # x free width per group tile (8192)
BW = G * 128           # band free width per group tile (2048)
H = XW // 2            # half-group x width (4096)

_compiled = None


def _build_program(repeat=1):
    nc = bacc.Bacc("TRN2", target_bir_lowering=False, debug=False)
    f32, f16 = mybir.dt.float32, mybir.dt.float16

    x_in = nc.declare_dram_parameter("x", [NG * 128, XW], f16, isOutput=False)
    a_in = nc.declare_dram_parameter("bandA", [NG * 128, BW], f16,
                                     isOutput=False)
    b_in = nc.declare_dram_parameter("bandB", [NG * 80, BW], f16,
                                     isOutput=False)
    out_ext = nc.declare_dram_parameter("out", [NG * 128, XW], f16,
                                        isOutput=True)

    with tile.TileContext(nc) as tc:
        with (
            tc.tile_pool(name="xg", bufs=3) as xpool,
            tc.tile_pool(name="ag", bufs=3) as apool,
            tc.tile_pool(name="bg", bufs=3) as bpool,
            tc.tile_pool(name="og", bufs=3) as opool,
            tc.tile_pool(name="ps", bufs=6, space="PSUM") as ppool,
        ):
            for g in range(NG * repeat):
                g = g % NG
                ag = apool.tile([128, BW], f16)
                nc.sync.dma_start(out=ag[:], in_=a_in[g * 128:(g + 1) * 128, :])
                bg = bpool.tile([80, BW], f16)
                nc.sync.dma_start(out=bg[:], in_=b_in[g * 80:(g + 1) * 80, :])
                xg = xpool.tile([128, XW], f16)
                rows = x_in[g * 128:(g + 1) * 128, :]
                nc.sync.dma_start(out=xg[:, 0:H], in_=rows[:, 0:H])
                nc.sync.dma_start(out=xg[:, H:XW], in_=rows[:, H:XW])
                og = opool.tile([128, XW], f16)
                for j in range(G):
                    ps = ppool.tile([128, 512], f32)
                    nc.tensor.matmul(
                        out=ps[:],
                        lhsT=ag[:, j * 128:(j + 1) * 128],
                        rhs=xg[:, j * 512:(j + 1) * 512],
                        start=True, stop=False)
                    nc.tensor.matmul(
                        out=ps[:, 0:480],
                        lhsT=bg[:, j * 128:(j + 1) * 128],
                        rhs=xg[0:80, j * 512 + 32:(j + 1) * 512],
                        start=False, stop=True)
                    dst = og[:, j * 512:(j + 1) * 512]
                    if j % 2 == 0:
                        nc.vector.tensor_copy(out=dst, in_=ps[:])
                    else:
                        nc.scalar.copy(out=dst, in_=ps[:])
                    if j == G // 2 - 1:
                        nc.scalar.dma_start(
                            out=out_ext[g * 128:(g + 1) * 128, 0:H],
                            in_=og[:, 0:H])
                nc.scalar.dma_start(
                    out=out_ext[g * 128:(g + 1) * 128, H:XW], in_=og[:, H:XW])
    nc.finalize()
    return nc


def _build_bands(weight):
    """Per-core grouped band tensors, fp16.

    bandA[core]: [NG*128, BW]  A_grp[m, j*128+p] = w16[f, m-p]
    bandB[core]: [NG*80,  BW]  B_grp[mr, j*128+p] = w16[f, mr+128-p]
    with f = core*FC + g*G + j, row blocks of 128 (A) / 80 (B) per group g.
    """
    w16 = weight.astype(np.float16)
    m = np.arange(128)[:, None]
    p = np.arange(128)[None, :]
    dA = m - p                          # [128, 128]
    mA = (dA >= 0) & (dA < K)
    iA = np.clip(dA, 0, K - 1)
    mr = np.arange(80)[:, None]
    dB = mr + 128 - p                   # [80, 128]
    mB = (dB >= 0) & (dB < K)
    iB = np.clip(dB, 0, K - 1)

    A = np.where(mA[None], w16[:, iA], np.float16(0))   # [F, 128, 128]
    Bm = np.where(mB[None], w16[:, iB], np.float16(0))  # [F, 80, 128]
    A = A.reshape(N_CORES, NG, G, 128, 128).transpose(0, 1, 3, 2, 4)
    A = np.ascontiguousarray(A).reshape(N_CORES, NG * 128, BW)
    Bm = Bm.reshape(N_CORES, NG, G, 80, 128).transpose(0, 1, 3, 2, 4)
    Bm = np.ascontiguousarray(Bm).reshape(N_CORES, NG * 80, BW)
    return A, Bm


def _build_x_panels(x):
    """Per-core x panels, fp16: [N_CORES, NG*128, XW] with
    panel[c, g*128+m, j*512 + i*32 + b] = x[i*128+m, b, c*FC + g*G + j]."""
    x16 = x.astype(np.float16)                       # [S, B, F]
    v = x16.reshape(NB, 128, B, N_CORES, NG, G)      # [i, m, b, c, g, j]
    v = v.transpose(3, 4, 1, 5, 0, 2)                # [c, g, m, j, i, b]
    return np.ascontiguousarray(v).reshape(N_CORES, NG * 128, XW)


def _unpack_out(outs):
    """outs: list of N_CORES arrays [NG*128, XW] fp16 -> [S, B, F] fp32."""
    o = np.stack(outs)                               # [c, NG*128, XW]
    o = o.reshape(N_CORES, NG, 128, G, NB, B)        # [c, g, m, j, i, b]
    o = o.transpose(4, 2, 5, 0, 1, 3)                # [i, m, b, c, g, j]
    return np.ascontiguousarray(o).reshape(S, B, F).astype(np.float32)


def _make_in_maps(x, weight):
    xp = _build_x_panels(np.asarray(x, dtype=np.float32))
    A, Bm = _build_bands(np.asarray(weight, dtype=np.float32))
    return [{"x": xp[c], "bandA": A[c], "bandB": Bm[c]}
            for c in range(N_CORES)]


def kernel(x, weight):
    global _compiled
    if _compiled is None:
        _compiled = _build_program()
    in_maps = _make_in_maps(x, weight)
    res = run_bass_kernel_spmd(_compiled, in_maps, list(range(N_CORES)))
    outs = [np.asarray(res.results[c]["out"]) for c in range(N_CORES)]
    return _unpack_out(outs)
